# revision 34
# baseline (speedup 1.0000x reference)
"""FastStackedSAE forward on 8 trn2 NeuronCores (Bass/Tile).

Strategy: tensor-parallel shard of d_sae (8192 -> 1024/core).
Per core: encode (f32 matmul, exact for top-k) -> relu -> local top-32
values per row via DVE max/match_replace -> AllGather candidates ->
global 32nd-largest threshold per row -> mask u = r * (r >= tau) (exact
in relu domain) -> decode partial x_hat in float32r (4x PE rate, ~1e-4
rel) -> ReduceScatter -> +b_dec, loss partial.  The pipeline runs in 4
quarters (2 t-slices each) so encode of quarter q+1 overlaps
threshold/mask/decode/collectives of quarter q.
Host: marshals transposed/sharded inputs, concatenates shard outputs,
sums 8 loss partials.
"""

import sys
from contextlib import ExitStack

sys.path.insert(0, "/opt/trn_rl_repo")

import numpy as np

import concourse.bass as bass
import concourse.mybir as mybir
import concourse.tile as tile
from concourse import bacc, bass_utils
from concourse.masks import make_identity

B, T, DIN, H, K = 256, 8, 768, 8192, 32
NCORES = 8
HS = H // NCORES          # 1024 features per core
ROWS = B * T              # 2048 rows, device order (t, b)
NT = ROWS // 128          # 16 row tiles, tile i -> t=i//2, rt=i%2
DC = DIN // 128           # 6 contraction chunks
HC = HS // 128            # 8 h-chunks per core
STAGES = (2, 2, 2, 2)     # t's per pipeline stage (sum = T)
NQ = len(STAGES)
T0 = [sum(STAGES[:i]) for i in range(NQ)]          # first t of each stage
FR = [256 * sz // NCORES for sz in STAGES]          # finalize rows per stage
FOFF = [sum(FR[:i]) for i in range(NQ)]             # piece offsets (sum = 256)
FTOT = sum(FR)                                      # 256
F32 = mybir.dt.float32
F32R = mybir.dt.float32r
NEG = -1.0                # sentinel below relu domain

_cache: dict = {}


def _emit_body(nc, tc, io, cn, rep, no_cc=False, use_benc=True, use_bdec=True):
    """Emit one full forward pass."""
    (xT, wencT, wdecT, bencS, bdec, xfin, bdfin, u_out, xhat_out, loss_out) = io
    ident, ones_k1, ones_col, bdec_sb = (
        cn["ident"], cn["ones_k1"], cn["ones_col"], cn["bdec_sb"]
    )
    R = f"_{rep}"
    rg = [list(range(NCORES))]

    with ExitStack() as ctx:
        dram_pool = ctx.enter_context(tc.tile_pool(name="dram" + R, bufs=1, space="DRAM"))
        rbuf = ctx.enter_context(tc.tile_pool(name="rbuf" + R, bufs=2 * max(STAGES[i] + STAGES[i + 1] for i in range(NQ - 1)) + 2))
        cand_pool = ctx.enter_context(tc.tile_pool(name="cand" + R, bufs=2))
        wenc_pool = ctx.enter_context(tc.tile_pool(name="wenc" + R, bufs=3))
        xc_pool = ctx.enter_context(tc.tile_pool(name="xc" + R, bufs=3))
        benc_pool = ctx.enter_context(tc.tile_pool(name="benc" + R, bufs=2))
        sc_pool = ctx.enter_context(tc.tile_pool(name="sc" + R, bufs=2))
        gcp = ctx.enter_context(tc.tile_pool(name="gcp" + R, bufs=5))
        mask_pool = ctx.enter_context(tc.tile_pool(name="mask" + R, bufs=2))
        wdec_pool = ctx.enter_context(tc.tile_pool(name="wdec" + R, bufs=8))
        ut_pool = ctx.enter_context(tc.tile_pool(name="ut" + R, bufs=2))
        xh_pool = ctx.enter_context(tc.tile_pool(name="xh" + R, bufs=2))
        fin_pool = ctx.enter_context(tc.tile_pool(name="fin" + R, bufs=1))
        psum_e_pool = ctx.enter_context(tc.tile_pool(name="psum_e" + R, bufs=2, space="PSUM"))
        psum_d_pool = ctx.enter_context(tc.tile_pool(name="psum_d" + R, bufs=2, space="PSUM"))
        psum_l_pool = ctx.enter_context(tc.tile_pool(name="psum_l" + R, bufs=1, space="PSUM"))
        rs_res = []
        psl = psum_l_pool.tile([1, 1], F32, name="psl" + R)

        state = {}

        def encode_q(q):
            Q = f"{R}q{q}"
            SZ = STAGES[q]
            # ---------------- ENCODE + LOCAL TOPK (t in stage q) ----------
            cand_q = cand_pool.tile([128, 2 * SZ, K], F32, tag="cand", name="cand" + Q)
            r_tiles = {}
            for tq in range(SZ):
                t = T0[q] + tq
                xc_t = xc_pool.tile([128, DC, B], F32, tag="xc", name=f"xc{R}_{t}")
                nc.sync.dma_start(xc_t[:], xT[t].rearrange("(o p) b -> p o b", p=128))
                if use_bdec:
                    for o in range(DC):
                        nc.vector.tensor_scalar(
                            xc_t[:, o, :], xc_t[:, o, :], bdec_sb[:, t, o : o + 1],
                            None, op0=mybir.AluOpType.subtract,
                        )
                if use_benc:
                    benc_t = benc_pool.tile([1, HS], F32, tag="benc", name=f"benc{R}_{t}")
                    nc.sync.dma_start(benc_t[:], bencS[None, t, :])

                w_t = [None, None]
                for hb in range(2):
                    w_t[hb] = wenc_pool.tile(
                        [128, DC, 512], F32, tag="wenc", name=f"wenc{R}_{t}_{hb}"
                    )
                    wsrc = wencT[t].rearrange("(o p) h -> p o h", p=128)
                    for o in range(DC):
                        nc.sync.dma_start(
                            w_t[hb][:, o, :],
                            wsrc[:, o, hb * 512 : (hb + 1) * 512],
                        )

                for rt in range(2):
                    i_q = tq * 2 + rt          # tile index within quarter (0..3)
                    r_i = rbuf.tile([128, HS], F32, tag="r", name=f"r{R}_{t}_{rt}")
                    for hb in range(2):
                        ps = psum_e_pool.tile(
                            [128, 512], F32, tag="ps", name=f"ps{R}_{t}_{rt}_{hb}"
                        )
                        for o in range(DC):
                            nc.tensor.matmul(
                                ps[:],
                                xc_t[:, o, rt * 128 : (rt + 1) * 128],
                                w_t[hb][:, o, :],
                                start=(o == 0),
                                stop=(o == DC - 1 and not use_benc),
                            )
                        if use_benc:
                            nc.tensor.matmul(
                                ps[:], ones_k1[:],
                                benc_t[:, hb * 512 : (hb + 1) * 512],
                                start=False, stop=True,
                            )
                        nc.scalar.activation(
                            r_i[:, hb * 512 : (hb + 1) * 512], ps[:],
                            mybir.ActivationFunctionType.Relu,
                        )
                    r_tiles[i_q] = r_i
                    # local top-32 values (sorted desc)
                    sc = sc_pool.tile([128, HS], F32, tag="sc", name=f"sc{R}_{t}_{rt}")
                    src = r_i
                    for j in range(4):
                        nc.vector.max(
                            out=cand_q[:, i_q, j * 8 : (j + 1) * 8], in_=src[:]
                        )
                        if j < 3:
                            nc.vector.match_replace(
                                out=sc[:],
                                in_to_replace=cand_q[:, i_q, j * 8 : (j + 1) * 8],
                                in_values=src[:],
                                imm_value=NEG,
                            )
                            src = sc

            # ---------------- ALLGATHER CANDIDATES (stage) ----------------
            cand_dram = dram_pool.tile([SZ * 256, K], F32, name="cand_dram" + Q)
            nc.sync.dma_start(
                cand_dram.rearrange("(i p) c -> p i c", p=128), cand_q[:]
            )
            gc_kw = {} if no_cc else {"addr_space": "Shared"}
            gcand = dram_pool.tile([NCORES, SZ * 256, K], F32, name="gcand" + Q, **gc_kw)
            if no_cc:
                for w in range(NCORES):
                    nc.sync.dma_start(gcand[w], cand_dram[:])
            else:
                nc.gpsimd.collective_compute(
                    "AllGather", mybir.AluOpType.bypass,
                    ins=[cand_dram[:].opt()], outs=[gcand[:].opt()],
                    replica_groups=rg,
                )
            state[q] = (r_tiles, gcand)

        def consume_q(q):
            Q = f"{R}q{q}"
            SZ = STAGES[q]
            r_tiles, gcand = state.pop(q)
            # -------- GLOBAL THRESHOLD + MASK + u + TRANSPOSE + DECODE ----
            xhp = dram_pool.tile([SZ * 256, DIN], F32, name="xhp" + Q)
            gcs = {}
            for tq in range(SZ):
                for rt in range(2):
                    i_q = tq * 2 + rt
                    gc = gcp.tile(
                        [128, NCORES * K], F32, tag="gc", name=f"gc{R}q{q}_{i_q}"
                    )
                    nc.sync.dma_start(
                        gc[:].rearrange("p (w c) -> p w c", w=NCORES),
                        gcand[:, i_q * 128 : (i_q + 1) * 128, :].rearrange(
                            "w p c -> p w c"
                        ),
                    )
                    gcs[i_q] = gc
            for tq in range(SZ):
                t = T0[q] + tq
                ut = ut_pool.tile([128, HC, B], F32R, tag="ut", name=f"ut{R}_{t}")
                wd = [None] * HC
                for hc in range(HC):
                    wd[hc] = wdec_pool.tile(
                        [128, DIN], F32R, tag="wd", name=f"wd{R}_{t}_{hc}"
                    )
                    nc.sync.dma_start(wd[hc][:], wdecT[t, hc * 128 : (hc + 1) * 128, :])
                for rt in range(2):
                    i_q = tq * 2 + rt
                    gc = gcs[i_q]
                    g32 = gcp.tile([128, K], F32, tag="g32", name=f"g32{R}_{t}_{rt}")
                    src = gc
                    for j in range(4):
                        nc.vector.max(out=g32[:, j * 8 : (j + 1) * 8], in_=src[:])
                        if j < 3:
                            nc.vector.match_replace(
                                out=gc[:], in_to_replace=g32[:, j * 8 : (j + 1) * 8],
                                in_values=src[:], imm_value=NEG,
                            )
                            src = gc
                    tau = g32[:, K - 1 : K]
                    r_i = r_tiles[i_q]
                    mask = mask_pool.tile([128, HS], F32, tag="mask", name=f"mask{R}_{t}_{rt}")
                    nc.vector.tensor_scalar(
                        mask[:], r_i[:], tau, None, op0=mybir.AluOpType.is_ge
                    )
                    nc.vector.tensor_tensor(
                        r_i[:], r_i[:], mask[:], op=mybir.AluOpType.mult
                    )
                    nc.sync.dma_start(u_out[rt * 128 : (rt + 1) * 128, t, :], r_i[:])

                    # transpose u -> uT (f32r) for decode
                    for hq in range(2):
                        pst = psum_e_pool.tile(
                            [128, 512], F32, tag="ps", name=f"pst{R}_{t}_{rt}_{hq}"
                        )
                        for c in range(4):
                            hc = hq * 4 + c
                            nc.tensor.transpose(
                                pst[:, c * 128 : (c + 1) * 128],
                                r_i[:, hc * 128 : (hc + 1) * 128],
                                ident[:],
                            )
                        for c in range(4):
                            hc = hq * 4 + c
                            nc.scalar.copy(
                                ut[:, hc, rt * 128 : (rt + 1) * 128],
                                pst[:, c * 128 : (c + 1) * 128],
                            )

                # decode for this t
                for rt in range(2):
                    psd = psum_d_pool.tile([128, DIN], F32, tag="psd", name=f"psd{R}_{t}_{rt}")
                    for hc in range(HC):
                        for n0, n1 in ((0, 512), (512, DIN)):
                            nc.tensor.matmul(
                                psd[:, n0:n1],
                                ut[:, hc, rt * 128 : (rt + 1) * 128],
                                wd[hc][:, n0:n1],
                                start=(hc == 0),
                                stop=(hc == HC - 1),
                            )
                    xh = xh_pool.tile([128, DIN], F32, tag="xh", name=f"xh{R}_{t}_{rt}")
                    nc.scalar.copy(xh[:], psd[:])
                    nc.sync.dma_start(
                        xhp[tq * 256 + rt * 128 : tq * 256 + rt * 128 + 128, :], xh[:]
                    )

            # ---------------- REDUCESCATTER (stage) -----------------------
            rs = dram_pool.tile([FR[q], DIN], F32, name=f"rs{R}_{q}")
            if no_cc:
                nc.sync.dma_start(rs[:], xhp[0 : FR[q], :])
            else:
                nc.gpsimd.collective_compute(
                    "ReduceScatter", mybir.AluOpType.add,
                    ins=[xhp[:].opt()], outs=[rs[:].opt()],
                    replica_groups=rg,
                )
            rs_res.append(rs)

        # ---------------- FINALIZE (one piece of FR[j] rows) -------------
        def finalize_q(j):
            FRj = FR[j]
            o0 = FOFF[j]
            rsb = fin_pool.tile([FRj, DIN], F32, tag="rsb", name=f"rsb{R}_{j}")
            nc.sync.dma_start(rsb[:], rs_res[j][:])
            if use_bdec:
                bdf = fin_pool.tile([1, DIN], F32, tag="bdf", name=f"bdf{R}_{j}")
                nc.sync.dma_start(bdf[:], bdfin[None, j, :])
                psf = psum_d_pool.tile([FRj, DIN], F32, tag="psd", name=f"psf{R}_{j}")
                for n0, n1 in ((0, 512), (512, DIN)):
                    nc.tensor.matmul(
                        psf[:, n0:n1], ones_k1[:, :FRj], bdf[:, n0:n1],
                        start=True, stop=True,
                    )
                xf = fin_pool.tile([FRj, DIN], F32, tag="xf", name=f"xf{R}_{j}")
                nc.vector.tensor_tensor(xf[:], rsb[:], psf[:], op=mybir.AluOpType.add)
            else:
                xf = rsb
            nc.sync.dma_start(xhat_out[o0 : o0 + FRj, :], xf[:])
            # loss partial
            xn = fin_pool.tile([FRj, DIN], F32, tag="xn", name=f"xn{R}_{j}")
            nc.sync.dma_start(xn[:], xfin[o0 : o0 + FRj, :])
            d = fin_pool.tile([FRj, DIN], F32, tag="d", name=f"d{R}_{j}")
            nc.vector.tensor_tensor(d[:], xf[:], xn[:], op=mybir.AluOpType.subtract)
            sq = fin_pool.tile([FRj, 1], F32, tag="sq", name=f"sq{R}_{j}")
            nc.scalar.activation(
                d[:], d[:], mybir.ActivationFunctionType.Square, accum_out=sq[:]
            )
            nc.tensor.matmul(
                psl[:], sq[:], ones_col[:FRj, :], start=(j == 0), stop=(j == NQ - 1)
            )

        # software pipeline: encode(q+1) is emitted before consume(q) so the
        # PE never waits on quarter q's AllGather/threshold round-trip;
        # finalize(j) follows consume(j+1) so it overlaps later quarters
        encode_q(0)
        for q in range(1, NQ):
            encode_q(q)
            consume_q(q - 1)
            if q >= 2:
                finalize_q(q - 2)
        consume_q(NQ - 1)
        finalize_q(NQ - 2)
        finalize_q(NQ - 1)

        lsb = fin_pool.tile([1, 1], F32, tag="lsb", name="lsb" + R)
        nc.scalar.activation(
            lsb[:], psl[0:1, :], mybir.ActivationFunctionType.Copy, scale=1.0 / ROWS
        )
        nc.sync.dma_start(loss_out[:], lsb[:])


def _build(reps=1, no_cc=False, use_benc=True, use_bdec=True):
    nc = bacc.Bacc(
        "TRN2", target_bir_lowering=False, debug=False,
        num_devices=1 if no_cc else NCORES,
    )

    # ---- per-core DRAM I/O ----
    xT = nc.dram_tensor("xT", [T, DIN, B], F32, kind="ExternalInput").ap()
    wencT = nc.dram_tensor("wencT", [T, DIN, HS], F32, kind="ExternalInput").ap()
    wdecT = nc.dram_tensor("wdecT", [T, HS, DIN], F32R, kind="ExternalInput").ap()
    bencS = nc.dram_tensor("bencS", [T, HS], F32, kind="ExternalInput").ap()
    bdec = nc.dram_tensor("bdec", [T, DIN], F32, kind="ExternalInput").ap()
    xfin = nc.dram_tensor("xfin", [FTOT, DIN], F32, kind="ExternalInput").ap()
    bdfin = nc.dram_tensor("bdfin", [NQ, DIN], F32, kind="ExternalInput").ap()

    u_out = nc.dram_tensor("u_out", [B, T, HS], F32, kind="ExternalOutput").ap()
    xhat_out = nc.dram_tensor("xhat_out", [FTOT, DIN], F32, kind="ExternalOutput").ap()
    loss_out = nc.dram_tensor("loss_out", [1, 1], F32, kind="ExternalOutput").ap()
    io = (xT, wencT, wdecT, bencS, bdec, xfin, bdfin, u_out, xhat_out, loss_out)

    with tile.TileContext(nc) as tc:
        with tc.tile_pool(name="consts", bufs=1) as consts:
            ident = consts.tile([128, 128], F32)
            make_identity(nc, ident[:])
            ones_k1 = consts.tile([1, 128], F32)
            nc.vector.memset(ones_k1[:], 1.0)
            ones_col = consts.tile([128, 1], F32)
            nc.vector.memset(ones_col[:], 1.0)
            bdec_sb = consts.tile([128, T, DC], F32)
            nc.sync.dma_start(bdec_sb[:], bdec.rearrange("t (o p) -> p t o", p=128))
            cn = {
                "ident": ident, "ones_k1": ones_k1, "ones_col": ones_col,
                "bdec_sb": bdec_sb,
            }
            for rep in range(reps):
                _emit_body(nc, tc, io, cn, rep, no_cc=no_cc,
                           use_benc=use_benc, use_bdec=use_bdec)

    nc.compile()
    return nc


def _get_nc(reps=1, no_cc=False, use_benc=True, use_bdec=True):
    key = f"nc{reps}_{no_cc}_{use_benc}_{use_bdec}"
    if key not in _cache:
        _cache[key] = _build(reps, no_cc=no_cc, use_benc=use_benc, use_bdec=use_bdec)
    return _cache[key]


def _fin_piece(s, j):
    """(t, b0, nrows) of finalize piece j on core s."""
    t = T0[j] + (s * FR[j]) // 256
    b0 = (s * FR[j]) % 256
    return t, b0, FR[j]


def make_in_maps(x, W_enc, b_enc, W_dec, b_dec, k):
    """Host-side sharding/marshalling: slice + transpose per core."""
    assert int(k) == K
    x = np.ascontiguousarray(x, dtype=np.float32)
    xT = np.ascontiguousarray(x.transpose(1, 2, 0))  # [T, DIN, B]
    bdec_c = np.ascontiguousarray(b_dec, dtype=np.float32)
    in_maps = []
    for s in range(NCORES):
        sl = slice(s * HS, (s + 1) * HS)
        wencT = np.ascontiguousarray(W_enc[:, sl, :].transpose(0, 2, 1))  # [T,DIN,HS]
        wdecT = np.ascontiguousarray(W_dec[:, :, sl].transpose(0, 2, 1))  # [T,HS,DIN]
        bencS = np.ascontiguousarray(b_enc[:, sl])
        pieces = [_fin_piece(s, j) for j in range(NQ)]
        xfin = np.concatenate([x[b0 : b0 + nr, t, :] for t, b0, nr in pieces])
        bdfin = np.stack([bdec_c[t] for t, b0, nr in pieces])
        in_maps.append(
            {
                "xT": xT,
                "wencT": wencT,
                "wdecT": wdecT,
                "bencS": bencS,
                "bdec": bdec_c,
                "xfin": np.ascontiguousarray(xfin),
                "bdfin": np.ascontiguousarray(bdfin),
            }
        )
    return in_maps


def assemble(results):
    """Host-side unshard: concat u/x_hat shards, sum loss partials."""
    u = np.concatenate([results[s]["u_out"] for s in range(NCORES)], axis=2)
    x_hat = np.empty((B, T, DIN), dtype=np.float32)
    loss = np.float32(0.0)
    for s in range(NCORES):
        xh = results[s]["xhat_out"]
        for j in range(NQ):
            t, b0, nr = _fin_piece(s, j)
            x_hat[b0 : b0 + nr, t, :] = xh[FOFF[j] : FOFF[j] + nr]
        loss += results[s]["loss_out"][0, 0]
    return np.float32(loss), x_hat, u


def kernel(x, W_enc, b_enc, W_dec, b_dec, k):
    use_benc = bool(np.any(b_enc))
    use_bdec = bool(np.any(b_dec))
    nc = _get_nc(use_benc=use_benc, use_bdec=use_bdec)
    in_maps = make_in_maps(x, W_enc, b_enc, W_dec, b_dec, k)
    last_err = None
    for attempt in range(3):
        try:
            res = bass_utils.run_bass_kernel_spmd(nc, in_maps, list(range(NCORES)))
            return assemble(res.results)
        except Exception as e:  # transient device hiccups recover after a pause
            last_err = e
            import time as _time

            _time.sleep(15)
    raise last_err


# revision 35
# speedup vs baseline: 1.0578x; 1.0578x over previous
"""FastStackedSAE forward on 8 trn2 NeuronCores (Bass/Tile).

Strategy: tensor-parallel shard of d_sae (8192 -> 1024/core).
Per core: encode (f32 matmul, exact for top-k) -> relu -> local top-32
values per row via DVE max/match_replace -> AllGather candidates ->
global 32nd-largest threshold per row -> mask u = r * (r >= tau) (exact
in relu domain) -> decode partial x_hat in float32r (4x PE rate, ~1e-4
rel) -> ReduceScatter -> +b_dec, loss partial.  The pipeline runs in 4
quarters (2 t-slices each) so encode of quarter q+1 overlaps
threshold/mask/decode/collectives of quarter q.
Host: marshals transposed/sharded inputs, concatenates shard outputs,
sums 8 loss partials.
"""

import sys
from contextlib import ExitStack

sys.path.insert(0, "/opt/trn_rl_repo")

import numpy as np

import concourse.bass as bass
import concourse.mybir as mybir
import concourse.tile as tile
from concourse import bacc, bass_utils
from concourse.masks import make_identity

B, T, DIN, H, K = 256, 8, 768, 8192, 32
NCORES = 8
HS = H // NCORES          # 1024 features per core
ROWS = B * T              # 2048 rows, device order (t, b)
NT = ROWS // 128          # 16 row tiles, tile i -> t=i//2, rt=i%2
DC = DIN // 128           # 6 contraction chunks
HC = HS // 128            # 8 h-chunks per core
STAGES = (2, 2, 2, 2)     # t's per pipeline stage (sum = T)
NQ = len(STAGES)
T0 = [sum(STAGES[:i]) for i in range(NQ)]          # first t of each stage
FR = [256 * sz // NCORES for sz in STAGES]          # finalize rows per stage
FOFF = [sum(FR[:i]) for i in range(NQ)]             # piece offsets (sum = 256)
FTOT = sum(FR)                                      # 256
F32 = mybir.dt.float32
F32R = mybir.dt.float32r
NEG = -1.0                # sentinel below relu domain

_cache: dict = {}


def _emit_body(nc, tc, io, cn, rep, no_cc=False, use_benc=True, use_bdec=True):
    """Emit one full forward pass."""
    (xT, wencT, wdecT, bencS, bdec, xfin, bdfin, u_out, xhat_out, loss_out) = io
    ident, ones_k1, ones_col, bdec_sb = (
        cn["ident"], cn["ones_k1"], cn["ones_col"], cn["bdec_sb"]
    )
    R = f"_{rep}"
    rg = [list(range(NCORES))]

    with ExitStack() as ctx:
        dram_pool = ctx.enter_context(tc.tile_pool(name="dram" + R, bufs=1, space="DRAM"))
        rbuf = ctx.enter_context(tc.tile_pool(name="rbuf" + R, bufs=2 * max(STAGES[i] + STAGES[i + 1] for i in range(NQ - 1)) + 2))
        cand_pool = ctx.enter_context(tc.tile_pool(name="cand" + R, bufs=2))
        wenc_pool = ctx.enter_context(tc.tile_pool(name="wenc" + R, bufs=3))
        xc_pool = ctx.enter_context(tc.tile_pool(name="xc" + R, bufs=3))
        benc_pool = ctx.enter_context(tc.tile_pool(name="benc" + R, bufs=2))
        sc_pool = ctx.enter_context(tc.tile_pool(name="sc" + R, bufs=2))
        gcp = ctx.enter_context(tc.tile_pool(name="gcp" + R, bufs=5))
        mask_pool = ctx.enter_context(tc.tile_pool(name="mask" + R, bufs=2))
        wdec_pool = ctx.enter_context(tc.tile_pool(name="wdec" + R, bufs=8))
        ut_pool = ctx.enter_context(tc.tile_pool(name="ut" + R, bufs=2))
        xh_pool = ctx.enter_context(tc.tile_pool(name="xh" + R, bufs=2))
        fin_pool = ctx.enter_context(tc.tile_pool(name="fin" + R, bufs=1))
        psum_e_pool = ctx.enter_context(tc.tile_pool(name="psum_e" + R, bufs=2, space="PSUM"))
        psum_d_pool = ctx.enter_context(tc.tile_pool(name="psum_d" + R, bufs=2, space="PSUM"))
        psum_l_pool = ctx.enter_context(tc.tile_pool(name="psum_l" + R, bufs=1, space="PSUM"))
        rs_res = []
        psl = psum_l_pool.tile([1, 1], F32, name="psl" + R)

        state = {}

        def encode_q(q):
            Q = f"{R}q{q}"
            SZ = STAGES[q]
            # ---------------- ENCODE + LOCAL TOPK (t in stage q) ----------
            cand_q = cand_pool.tile([128, 2 * SZ, K], F32, tag="cand", name="cand" + Q)
            r_tiles = {}
            for tq in range(SZ):
                t = T0[q] + tq
                xc_t = xc_pool.tile([128, DC, B], F32, tag="xc", name=f"xc{R}_{t}")
                xsrc = xT[t].rearrange("(o p) b -> p o b", p=128)
                for o in range(DC):
                    nc.sync.dma_start(xc_t[:, o, :], xsrc[:, o, :])
                if use_bdec:
                    for o in range(DC):
                        nc.vector.tensor_scalar(
                            xc_t[:, o, :], xc_t[:, o, :], bdec_sb[:, t, o : o + 1],
                            None, op0=mybir.AluOpType.subtract,
                        )
                if use_benc:
                    benc_t = benc_pool.tile([1, HS], F32, tag="benc", name=f"benc{R}_{t}")
                    nc.sync.dma_start(benc_t[:], bencS[None, t, :])

                w_t = [None, None]
                for hb in range(2):
                    w_t[hb] = wenc_pool.tile(
                        [128, DC, 512], F32, tag="wenc", name=f"wenc{R}_{t}_{hb}"
                    )
                    wsrc = wencT[t].rearrange("(o p) h -> p o h", p=128)
                    for o in range(DC):
                        nc.sync.dma_start(
                            w_t[hb][:, o, :],
                            wsrc[:, o, hb * 512 : (hb + 1) * 512],
                        )

                for rt in range(2):
                    i_q = tq * 2 + rt          # tile index within quarter (0..3)
                    r_i = rbuf.tile([128, HS], F32, tag="r", name=f"r{R}_{t}_{rt}")
                    for hb in range(2):
                        ps = psum_e_pool.tile(
                            [128, 512], F32, tag="ps", name=f"ps{R}_{t}_{rt}_{hb}"
                        )
                        for o in range(DC):
                            nc.tensor.matmul(
                                ps[:],
                                xc_t[:, o, rt * 128 : (rt + 1) * 128],
                                w_t[hb][:, o, :],
                                start=(o == 0),
                                stop=(o == DC - 1 and not use_benc),
                            )
                        if use_benc:
                            nc.tensor.matmul(
                                ps[:], ones_k1[:],
                                benc_t[:, hb * 512 : (hb + 1) * 512],
                                start=False, stop=True,
                            )
                        nc.scalar.activation(
                            r_i[:, hb * 512 : (hb + 1) * 512], ps[:],
                            mybir.ActivationFunctionType.Relu,
                        )
                    r_tiles[i_q] = r_i
                    # local top-32 values (sorted desc)
                    sc = sc_pool.tile([128, HS], F32, tag="sc", name=f"sc{R}_{t}_{rt}")
                    src = r_i
                    for j in range(4):
                        nc.vector.max(
                            out=cand_q[:, i_q, j * 8 : (j + 1) * 8], in_=src[:]
                        )
                        if j < 3:
                            nc.vector.match_replace(
                                out=sc[:],
                                in_to_replace=cand_q[:, i_q, j * 8 : (j + 1) * 8],
                                in_values=src[:],
                                imm_value=NEG,
                            )
                            src = sc

            # ---------------- ALLGATHER CANDIDATES (stage) ----------------
            cand_dram = dram_pool.tile([SZ * 256, K], F32, name="cand_dram" + Q)
            nc.sync.dma_start(
                cand_dram.rearrange("(i p) c -> p i c", p=128), cand_q[:]
            )
            gc_kw = {} if no_cc else {"addr_space": "Shared"}
            gcand = dram_pool.tile([NCORES, SZ * 256, K], F32, name="gcand" + Q, **gc_kw)
            if no_cc:
                for w in range(NCORES):
                    nc.sync.dma_start(gcand[w], cand_dram[:])
            else:
                nc.gpsimd.collective_compute(
                    "AllGather", mybir.AluOpType.bypass,
                    ins=[cand_dram[:].opt()], outs=[gcand[:].opt()],
                    replica_groups=rg,
                )
            state[q] = (r_tiles, gcand)

        def consume_q(q):
            Q = f"{R}q{q}"
            SZ = STAGES[q]
            r_tiles, gcand = state.pop(q)
            # -------- GLOBAL THRESHOLD + MASK + u + TRANSPOSE + DECODE ----
            xhp = dram_pool.tile([SZ * 256, DIN], F32, name="xhp" + Q)
            gcs = {}
            for tq in range(SZ):
                for rt in range(2):
                    i_q = tq * 2 + rt
                    gc = gcp.tile(
                        [128, NCORES * K], F32, tag="gc", name=f"gc{R}q{q}_{i_q}"
                    )
                    nc.sync.dma_start(
                        gc[:].rearrange("p (w c) -> p w c", w=NCORES),
                        gcand[:, i_q * 128 : (i_q + 1) * 128, :].rearrange(
                            "w p c -> p w c"
                        ),
                    )
                    gcs[i_q] = gc
            for tq in range(SZ):
                t = T0[q] + tq
                ut = ut_pool.tile([128, HC, B], F32R, tag="ut", name=f"ut{R}_{t}")
                wd = [None] * HC
                for hc in range(HC):
                    wd[hc] = wdec_pool.tile(
                        [128, DIN], F32R, tag="wd", name=f"wd{R}_{t}_{hc}"
                    )
                    nc.sync.dma_start(wd[hc][:], wdecT[t, hc * 128 : (hc + 1) * 128, :])
                for rt in range(2):
                    i_q = tq * 2 + rt
                    gc = gcs[i_q]
                    g32 = gcp.tile([128, K], F32, tag="g32", name=f"g32{R}_{t}_{rt}")
                    src = gc
                    for j in range(4):
                        nc.vector.max(out=g32[:, j * 8 : (j + 1) * 8], in_=src[:])
                        if j < 3:
                            nc.vector.match_replace(
                                out=gc[:], in_to_replace=g32[:, j * 8 : (j + 1) * 8],
                                in_values=src[:], imm_value=NEG,
                            )
                            src = gc
                    tau = g32[:, K - 1 : K]
                    r_i = r_tiles[i_q]
                    mask = mask_pool.tile([128, HS], F32, tag="mask", name=f"mask{R}_{t}_{rt}")
                    nc.vector.tensor_scalar(
                        mask[:], r_i[:], tau, None, op0=mybir.AluOpType.is_ge
                    )
                    nc.vector.tensor_tensor(
                        r_i[:], r_i[:], mask[:], op=mybir.AluOpType.mult
                    )
                    nc.sync.dma_start(u_out[rt * 128 : (rt + 1) * 128, t, :], r_i[:])

                    # transpose u -> uT (f32r) for decode
                    for hq in range(2):
                        pst = psum_e_pool.tile(
                            [128, 512], F32, tag="ps", name=f"pst{R}_{t}_{rt}_{hq}"
                        )
                        for c in range(4):
                            hc = hq * 4 + c
                            nc.tensor.transpose(
                                pst[:, c * 128 : (c + 1) * 128],
                                r_i[:, hc * 128 : (hc + 1) * 128],
                                ident[:],
                            )
                        for c in range(4):
                            hc = hq * 4 + c
                            nc.scalar.copy(
                                ut[:, hc, rt * 128 : (rt + 1) * 128],
                                pst[:, c * 128 : (c + 1) * 128],
                            )

                # decode for this t
                for rt in range(2):
                    psd = psum_d_pool.tile([128, DIN], F32, tag="psd", name=f"psd{R}_{t}_{rt}")
                    for hc in range(HC):
                        for n0, n1 in ((0, 512), (512, DIN)):
                            nc.tensor.matmul(
                                psd[:, n0:n1],
                                ut[:, hc, rt * 128 : (rt + 1) * 128],
                                wd[hc][:, n0:n1],
                                start=(hc == 0),
                                stop=(hc == HC - 1),
                            )
                    xh = xh_pool.tile([128, DIN], F32, tag="xh", name=f"xh{R}_{t}_{rt}")
                    nc.scalar.copy(xh[:], psd[:])
                    nc.sync.dma_start(
                        xhp[tq * 256 + rt * 128 : tq * 256 + rt * 128 + 128, :], xh[:]
                    )

            # ---------------- REDUCESCATTER (stage) -----------------------
            rs = dram_pool.tile([FR[q], DIN], F32, name=f"rs{R}_{q}")
            if no_cc:
                nc.sync.dma_start(rs[:], xhp[0 : FR[q], :])
            else:
                nc.gpsimd.collective_compute(
                    "ReduceScatter", mybir.AluOpType.add,
                    ins=[xhp[:].opt()], outs=[rs[:].opt()],
                    replica_groups=rg,
                )
            rs_res.append(rs)

        # ---------------- FINALIZE (one piece of FR[j] rows) -------------
        def finalize_q(j):
            FRj = FR[j]
            o0 = FOFF[j]
            rsb = fin_pool.tile([FRj, DIN], F32, tag="rsb", name=f"rsb{R}_{j}")
            nc.sync.dma_start(rsb[:], rs_res[j][:])
            if use_bdec:
                bdf = fin_pool.tile([1, DIN], F32, tag="bdf", name=f"bdf{R}_{j}")
                nc.sync.dma_start(bdf[:], bdfin[None, j, :])
                psf = psum_d_pool.tile([FRj, DIN], F32, tag="psd", name=f"psf{R}_{j}")
                for n0, n1 in ((0, 512), (512, DIN)):
                    nc.tensor.matmul(
                        psf[:, n0:n1], ones_k1[:, :FRj], bdf[:, n0:n1],
                        start=True, stop=True,
                    )
                xf = fin_pool.tile([FRj, DIN], F32, tag="xf", name=f"xf{R}_{j}")
                nc.vector.tensor_tensor(xf[:], rsb[:], psf[:], op=mybir.AluOpType.add)
            else:
                xf = rsb
            nc.sync.dma_start(xhat_out[o0 : o0 + FRj, :], xf[:])
            # loss partial
            xn = fin_pool.tile([FRj, DIN], F32, tag="xn", name=f"xn{R}_{j}")
            nc.sync.dma_start(xn[:], xfin[o0 : o0 + FRj, :])
            d = fin_pool.tile([FRj, DIN], F32, tag="d", name=f"d{R}_{j}")
            nc.vector.tensor_tensor(d[:], xf[:], xn[:], op=mybir.AluOpType.subtract)
            sq = fin_pool.tile([FRj, 1], F32, tag="sq", name=f"sq{R}_{j}")
            nc.scalar.activation(
                d[:], d[:], mybir.ActivationFunctionType.Square, accum_out=sq[:]
            )
            nc.tensor.matmul(
                psl[:], sq[:], ones_col[:FRj, :], start=(j == 0), stop=(j == NQ - 1)
            )

        # software pipeline: encode(q+1) is emitted before consume(q) so the
        # PE never waits on quarter q's AllGather/threshold round-trip;
        # finalize(j) follows consume(j+1) so it overlaps later quarters
        encode_q(0)
        for q in range(1, NQ):
            encode_q(q)
            consume_q(q - 1)
            if q >= 2:
                finalize_q(q - 2)
        consume_q(NQ - 1)
        finalize_q(NQ - 2)
        finalize_q(NQ - 1)

        lsb = fin_pool.tile([1, 1], F32, tag="lsb", name="lsb" + R)
        nc.scalar.activation(
            lsb[:], psl[0:1, :], mybir.ActivationFunctionType.Copy, scale=1.0 / ROWS
        )
        nc.sync.dma_start(loss_out[:], lsb[:])


def _build(reps=1, no_cc=False, use_benc=True, use_bdec=True):
    nc = bacc.Bacc(
        "TRN2", target_bir_lowering=False, debug=False,
        num_devices=1 if no_cc else NCORES,
    )

    # ---- per-core DRAM I/O ----
    xT = nc.dram_tensor("xT", [T, DIN, B], F32, kind="ExternalInput").ap()
    wencT = nc.dram_tensor("wencT", [T, DIN, HS], F32, kind="ExternalInput").ap()
    wdecT = nc.dram_tensor("wdecT", [T, HS, DIN], F32R, kind="ExternalInput").ap()
    bencS = nc.dram_tensor("bencS", [T, HS], F32, kind="ExternalInput").ap()
    bdec = nc.dram_tensor("bdec", [T, DIN], F32, kind="ExternalInput").ap()
    xfin = nc.dram_tensor("xfin", [FTOT, DIN], F32, kind="ExternalInput").ap()
    bdfin = nc.dram_tensor("bdfin", [NQ, DIN], F32, kind="ExternalInput").ap()

    u_out = nc.dram_tensor("u_out", [B, T, HS], F32, kind="ExternalOutput").ap()
    xhat_out = nc.dram_tensor("xhat_out", [FTOT, DIN], F32, kind="ExternalOutput").ap()
    loss_out = nc.dram_tensor("loss_out", [1, 1], F32, kind="ExternalOutput").ap()
    io = (xT, wencT, wdecT, bencS, bdec, xfin, bdfin, u_out, xhat_out, loss_out)

    with tile.TileContext(nc) as tc:
        with tc.tile_pool(name="consts", bufs=1) as consts:
            ident = consts.tile([128, 128], F32)
            make_identity(nc, ident[:])
            ones_k1 = consts.tile([1, 128], F32)
            nc.vector.memset(ones_k1[:], 1.0)
            ones_col = consts.tile([128, 1], F32)
            nc.vector.memset(ones_col[:], 1.0)
            bdec_sb = consts.tile([128, T, DC], F32)
            nc.sync.dma_start(bdec_sb[:], bdec.rearrange("t (o p) -> p t o", p=128))
            cn = {
                "ident": ident, "ones_k1": ones_k1, "ones_col": ones_col,
                "bdec_sb": bdec_sb,
            }
            for rep in range(reps):
                _emit_body(nc, tc, io, cn, rep, no_cc=no_cc,
                           use_benc=use_benc, use_bdec=use_bdec)

    nc.compile()
    return nc


def _get_nc(reps=1, no_cc=False, use_benc=True, use_bdec=True):
    key = f"nc{reps}_{no_cc}_{use_benc}_{use_bdec}"
    if key not in _cache:
        _cache[key] = _build(reps, no_cc=no_cc, use_benc=use_benc, use_bdec=use_bdec)
    return _cache[key]


def _fin_piece(s, j):
    """(t, b0, nrows) of finalize piece j on core s."""
    t = T0[j] + (s * FR[j]) // 256
    b0 = (s * FR[j]) % 256
    return t, b0, FR[j]


def make_in_maps(x, W_enc, b_enc, W_dec, b_dec, k):
    """Host-side sharding/marshalling: slice + transpose per core."""
    assert int(k) == K
    x = np.ascontiguousarray(x, dtype=np.float32)
    xT = np.ascontiguousarray(x.transpose(1, 2, 0))  # [T, DIN, B]
    bdec_c = np.ascontiguousarray(b_dec, dtype=np.float32)
    in_maps = []
    for s in range(NCORES):
        sl = slice(s * HS, (s + 1) * HS)
        wencT = np.ascontiguousarray(W_enc[:, sl, :].transpose(0, 2, 1))  # [T,DIN,HS]
        wdecT = np.ascontiguousarray(W_dec[:, :, sl].transpose(0, 2, 1))  # [T,HS,DIN]
        bencS = np.ascontiguousarray(b_enc[:, sl])
        pieces = [_fin_piece(s, j) for j in range(NQ)]
        xfin = np.concatenate([x[b0 : b0 + nr, t, :] for t, b0, nr in pieces])
        bdfin = np.stack([bdec_c[t] for t, b0, nr in pieces])
        in_maps.append(
            {
                "xT": xT,
                "wencT": wencT,
                "wdecT": wdecT,
                "bencS": bencS,
                "bdec": bdec_c,
                "xfin": np.ascontiguousarray(xfin),
                "bdfin": np.ascontiguousarray(bdfin),
            }
        )
    return in_maps


def assemble(results):
    """Host-side unshard: concat u/x_hat shards, sum loss partials."""
    u = np.concatenate([results[s]["u_out"] for s in range(NCORES)], axis=2)
    x_hat = np.empty((B, T, DIN), dtype=np.float32)
    loss = np.float32(0.0)
    for s in range(NCORES):
        xh = results[s]["xhat_out"]
        for j in range(NQ):
            t, b0, nr = _fin_piece(s, j)
            x_hat[b0 : b0 + nr, t, :] = xh[FOFF[j] : FOFF[j] + nr]
        loss += results[s]["loss_out"][0, 0]
    return np.float32(loss), x_hat, u


def kernel(x, W_enc, b_enc, W_dec, b_dec, k):
    use_benc = bool(np.any(b_enc))
    use_bdec = bool(np.any(b_dec))
    nc = _get_nc(use_benc=use_benc, use_bdec=use_bdec)
    in_maps = make_in_maps(x, W_enc, b_enc, W_dec, b_dec, k)
    last_err = None
    for attempt in range(3):
        try:
            res = bass_utils.run_bass_kernel_spmd(nc, in_maps, list(range(NCORES)))
            return assemble(res.results)
        except Exception as e:  # transient device hiccups recover after a pause
            last_err = e
            import time as _time

            _time.sleep(15)
    raise last_err
